# revision 4
# baseline (speedup 1.0000x reference)
"""SAGAN-style attention block (nn_AttentionBlock) on 8 Trainium2 NeuronCores.

Math (per batch b):
    q = wq @ x + bq            [C8, N]
    k = wk @ x + bk            [C8, N]
    v = wv @ x + bv            [C,  N]
    S[n, m]  = sum_o q[o,n] k[o,m]
    attn     = softmax_m(S)
    out[c,n] = sum_m v[c,m] attn[n,m]
    y        = gamma * out + x

Sharding: 8 cores = 4 batches x 2 halves of the n (query-row) axis.  Each
core holds the full x[b] (for K/V) plus its own n-slice (for Q + residual).

Per-core layout (everything channels/rows on partitions):
    S^T tiles [m(128 part), n(512)] via matmul(lhsT=k_tile, rhs=q_block)
    P^T = exp(S^T)  (no max subtraction: |S| <~ 40, safe in fp32/bf16)
    out[c,n]   = sum over 32 m-tiles of matmul(lhsT=vT[m,c], rhs=P^T[m,n])
    denom[1,n] = matmul(lhsT=ones[128,1], rhs=P^T)  accumulated the same way
    y = out * (gamma/denom broadcast) + x

Matmuls run as float32r (full-rate fp32 PE mode); the AV/denominator path
runs in bf16 (P^T is written by the Exp activation directly as bf16).
"""

import sys

sys.path.insert(0, "/opt/trn_rl_repo")

import numpy as np  # noqa: E402

B, C, HH, WW = 4, 256, 64, 64
N = HH * WW  # 4096
C8 = C // 8  # 32
P = 128
CT = C // P  # 2 channel tiles
NQ = N // 2  # 2048 query rows per core
NBLK = 512  # n-block (query columns per block)
NBLKS = NQ // NBLK  # 4
MT = N // P  # 32 m-tiles (key/value positions)
CHUNK = 512
NCHUNKS = N // CHUNK  # 8
QCHUNKS = NQ // CHUNK  # 4
NCORES = 8

_prog = None


def _build():
    import concourse.bacc as bacc
    import concourse.mybir as mybir
    import concourse.tile as tile

    f32 = mybir.dt.float32
    f32r = mybir.dt.float32r
    bf16 = mybir.dt.bfloat16
    Exp = mybir.ActivationFunctionType.Exp

    nc = bacc.Bacc("TRN2", target_bir_lowering=False, debug=False)

    xh_d = nc.dram_tensor("xh", [C, N], f32r, kind="ExternalInput")
    xq_d = nc.dram_tensor("xq", [C, NQ], f32r, kind="ExternalInput")
    # wqt/wkt are zero-padded on the host from [C, C8] to [C, 128] so the
    # projection matmul writes all 128 partitions of q/k (zero rows included)
    wqt_d = nc.dram_tensor("wqt", [C, P], f32r, kind="ExternalInput")
    wkt_d = nc.dram_tensor("wkt", [C, P], f32r, kind="ExternalInput")
    wvt_d = nc.dram_tensor("wvt", [C, C], f32r, kind="ExternalInput")
    bq_d = nc.dram_tensor("bq", [P], f32, kind="ExternalInput")
    bk_d = nc.dram_tensor("bk", [P], f32, kind="ExternalInput")
    bv_d = nc.dram_tensor("bv", [C], f32, kind="ExternalInput")
    g_d = nc.dram_tensor("gamma", [1], f32, kind="ExternalInput")
    out_d = nc.dram_tensor("out", [C, NQ], f32, kind="ExternalOutput")

    with tile.TileContext(nc) as tc:
        with (
            tc.tile_pool(name="const", bufs=1) as const,
            tc.tile_pool(name="big", bufs=1) as big,
        ):
            # persistent SBUF tensors
            xh = big.tile([P, CT, N], f32r)  # x[b], channels on partitions
            xq = big.tile([P, CT, NQ], f32r)  # this core's n-slice of x[b]
            k_sb = big.tile([P, N], f32r)  # k, zero-padded to 128 partitions
            q_sb = big.tile([P, NQ], f32r)  # q, zero-padded
            vt_sb = big.tile([P, MT, C], bf16)  # v^T tiles [m, c]

            wqt = const.tile([P, CT, P], f32r)
            wkt = const.tile([P, CT, P], f32r)
            wvt = const.tile([P, CT, C], f32r)
            bq_sb = const.tile([P, 1], f32)
            bk_sb = const.tile([P, 1], f32)
            bvb = const.tile([P, C], f32)  # bv broadcast over partitions
            gam = const.tile([1, 1], f32)
            ones_bf = const.tile([P, 1], bf16)
            ones1 = const.tile([1, P], f32)

            nc.sync.dma_start(out=wqt, in_=wqt_d.ap().rearrange("(t p) o -> p t o", p=P))
            nc.sync.dma_start(out=wkt, in_=wkt_d.ap().rearrange("(t p) o -> p t o", p=P))
            nc.sync.dma_start(out=wvt, in_=wvt_d.ap().rearrange("(t p) o -> p t o", p=P))
            nc.sync.dma_start(out=bq_sb, in_=bq_d.ap()[:, None])
            nc.sync.dma_start(out=bk_sb, in_=bk_d.ap()[:, None])
            nc.gpsimd.dma_start(out=bvb, in_=bv_d.ap()[None, :].to_broadcast([P, C]))
            nc.sync.dma_start(out=gam, in_=g_d.ap()[:, None])
            nc.vector.memset(ones_bf, 1.0)
            nc.vector.memset(ones1, 1.0)

            xh_r = xh_d.ap().rearrange("(t p) n -> p t n", p=P)
            xq_r = xq_d.ap().rearrange("(t p) n -> p t n", p=P)
            out_r = out_d.ap().rearrange("(t p) n -> p t n", p=P)

            for ch in range(NCHUNKS):
                sl = slice(ch * CHUNK, (ch + 1) * CHUNK)
                nc.sync.dma_start(out=xh[:, :, sl], in_=xh_r[:, :, sl])
            for ch in range(QCHUNKS):
                sl = slice(ch * CHUNK, (ch + 1) * CHUNK)
                nc.sync.dma_start(out=xq[:, :, sl], in_=xq_r[:, :, sl])

            # ---- phase A: q/k/v projections ----
            with tc.tile_pool(name="pa_psum", bufs=2, space="PSUM") as pap:
                for ch in range(NCHUNKS):
                    sl = slice(ch * CHUNK, (ch + 1) * CHUNK)
                    kp = pap.tile([P, CHUNK], f32, tag="kq", name="kp")
                    for t in range(CT):
                        nc.tensor.matmul(
                            kp,
                            lhsT=wkt[:, t, :],
                            rhs=xh[:, t, sl],
                            start=(t == 0),
                            stop=(t == CT - 1),
                        )
                    nc.vector.tensor_scalar_add(k_sb[:, sl], kp, bk_sb)
                for ch in range(QCHUNKS):
                    sl = slice(ch * CHUNK, (ch + 1) * CHUNK)
                    qp = pap.tile([P, CHUNK], f32, tag="kq", name="qp")
                    for t in range(CT):
                        nc.tensor.matmul(
                            qp,
                            lhsT=wqt[:, t, :],
                            rhs=xq[:, t, sl],
                            start=(t == 0),
                            stop=(t == CT - 1),
                        )
                    nc.vector.tensor_scalar_add(q_sb[:, sl], qp, bq_sb)
                for mt in range(MT):
                    msl = slice(mt * P, (mt + 1) * P)
                    vp = pap.tile([P, C], f32, tag="v", name="vp")
                    for t in range(CT):
                        nc.tensor.matmul(
                            vp,
                            lhsT=xh[:, t, msl],
                            rhs=wvt[:, t, :],
                            start=(t == 0),
                            stop=(t == CT - 1),
                        )
                    # drain + bias + cast to bf16 in one DVE op
                    nc.vector.tensor_add(out=vt_sb[:, mt, :], in0=vp, in1=bvb)

            # ---- phase B: attention ----
            GRP = 2  # m-tiles per S^T psum group (2 banks)
            with (
                tc.tile_pool(name="st_psum", bufs=2, space="PSUM") as stp,
                tc.tile_pool(name="acc_psum", bufs=1, space="PSUM") as accp,
                tc.tile_pool(name="pt_pool", bufs=2) as ptp,
                tc.tile_pool(name="fin_pool", bufs=3) as finp,
            ):
                for nb in range(NBLKS):
                    nsl = slice(nb * NBLK, (nb + 1) * NBLK)
                    pt = ptp.tile([P, MT, NBLK], bf16, tag="pt", name="pt")
                    out_ps0 = accp.tile([P, NBLK], f32, tag="out0", name="out_ps0")
                    out_ps1 = accp.tile([P, NBLK], f32, tag="out1", name="out_ps1")
                    out_ps = [out_ps0, out_ps1]
                    den_ps = accp.tile([1, NBLK], f32, tag="den", name="den_ps")
                    for mg in range(MT // GRP):
                        st = stp.tile([P, GRP, NBLK], f32, tag="st", name="st")
                        for i in range(GRP):
                            mt = GRP * mg + i
                            nc.tensor.matmul(
                                st[:, i, :],
                                lhsT=k_sb[:, mt * P : (mt + 1) * P],
                                rhs=q_sb[:, nsl],
                                start=True,
                                stop=True,
                            )
                        nc.scalar.activation(
                            out=pt[:, GRP * mg : GRP * (mg + 1), :], in_=st, func=Exp
                        )
                        for i in range(GRP):
                            mt = GRP * mg + i
                            for cc in range(CT):
                                nc.tensor.matmul(
                                    out_ps[cc],
                                    lhsT=vt_sb[:, mt, cc * P : (cc + 1) * P],
                                    rhs=pt[:, mt, :],
                                    start=(mt == 0),
                                    stop=(mt == MT - 1),
                                )
                            nc.tensor.matmul(
                                den_ps,
                                lhsT=ones_bf,
                                rhs=pt[:, mt, :],
                                start=(mt == 0),
                                stop=(mt == MT - 1),
                            )
                    # normalization: y = out * (gamma/denom) + x
                    rec = finp.tile([1, NBLK], f32, tag="rec", name="rec")
                    nc.vector.reciprocal(rec, den_ps)
                    nc.vector.tensor_scalar_mul(rec, rec, gam)
                    bc_ps = accp.tile([P, NBLK], f32, tag="bc", name="bc_ps")
                    # exact fp32 outer-product broadcast: bc[p, n] = rec[n]
                    nc.tensor.matmul(bc_ps, lhsT=ones1, rhs=rec, start=True, stop=True)
                    bc_sb = finp.tile([P, NBLK], f32, tag="bcs", name="bc_sb")
                    nc.vector.tensor_copy(out=bc_sb, in_=bc_ps)
                    for cc in range(CT):
                        fin = finp.tile([P, NBLK], f32, tag="fin", name="fin")
                        nc.vector.tensor_mul(out=fin, in0=out_ps[cc], in1=bc_sb)
                        nc.vector.tensor_add(out=fin, in0=fin, in1=xq[:, cc, nsl].bitcast(f32))
                        nc.sync.dma_start(out=out_r[:, cc, nsl], in_=fin)

    nc.compile()
    return nc


def _get_prog():
    global _prog
    if _prog is None:
        _prog = _build()
    return _prog


def make_in_maps(inputs):
    x = np.ascontiguousarray(inputs["x"], dtype=np.float32).reshape(B, C, N)
    wqt = np.zeros((C, P), np.float32)
    wqt[:, :C8] = np.asarray(inputs["wq"], np.float32).T
    wkt = np.zeros((C, P), np.float32)
    wkt[:, :C8] = np.asarray(inputs["wk"], np.float32).T
    wvt = np.ascontiguousarray(np.asarray(inputs["wv"], np.float32).T)
    bq = np.zeros(P, np.float32)
    bq[:C8] = np.asarray(inputs["bq"], np.float32)
    bk = np.zeros(P, np.float32)
    bk[:C8] = np.asarray(inputs["bk"], np.float32)
    bv = np.ascontiguousarray(np.asarray(inputs["bv"], np.float32))
    gamma = np.ascontiguousarray(np.asarray(inputs["gamma"], np.float32).reshape(1))
    in_maps = []
    for core in range(NCORES):
        b, h = divmod(core, 2)
        in_maps.append(
            {
                "xh": x[b],
                "xq": np.ascontiguousarray(x[b][:, h * NQ : (h + 1) * NQ]),
                "wqt": wqt,
                "wkt": wkt,
                "wvt": wvt,
                "bq": bq,
                "bk": bk,
                "bv": bv,
                "gamma": gamma,
            }
        )
    return in_maps


def assemble(results):
    out = np.empty((B, C, N), np.float32)
    for core in range(NCORES):
        b, h = divmod(core, 2)
        out[b][:, h * NQ : (h + 1) * NQ] = results[core]["out"]
    return out.reshape(B, C, HH, WW)


def kernel(**inputs):
    from concourse.bass_utils import run_bass_kernel_spmd

    nc = _get_prog()
    in_maps = make_in_maps(inputs)
    res = run_bass_kernel_spmd(nc, in_maps, core_ids=list(range(NCORES)))
    return assemble(res.results)


# revision 5
# speedup vs baseline: 974.5319x; 974.5319x over previous
"""SAGAN-style attention block (nn_AttentionBlock) on 8 Trainium2 NeuronCores.

Math (per batch b):
    q = wq @ x + bq            [C8, N]
    k = wk @ x + bk            [C8, N]
    v = wv @ x + bv            [C,  N]
    S[n, m]  = sum_o q[o,n] k[o,m]
    attn     = softmax_m(S)
    out[c,n] = sum_m v[c,m] attn[n,m]
    y        = gamma * out + x

Sharding: 8 cores = 4 batches x 2 halves of the n (query-row) axis.  Each
core holds the full x[b] (for K/V) plus its own n-slice (for Q + residual).

Per-core layout (channels/rows on partitions):
    S^T tiles [m(128 part), n(512)] via matmul(lhsT=k_tile, rhs=q_block)
    P^T = exp(S^T)  (no max subtraction: |S| <~ 40, safe in fp32/bf16)
    out[c,n]   = sum over 32 m-tiles of matmul(lhsT=vT[m,c], rhs=P^T[m,n])
    denom[1,n] = matmul(lhsT=ones[128,1], rhs=P^T)  accumulated the same way
    y = out * (gamma/denom broadcast) + x

QK^T/projection matmuls run as float32r (full-rate fp32 PE mode); the
AV/denominator path runs in bf16 (P^T is written by Exp directly as bf16).
"""

import sys

sys.path.insert(0, "/opt/trn_rl_repo")

import numpy as np  # noqa: E402

B, C, HH, WW = 4, 256, 64, 64
N = HH * WW  # 4096
C8 = C // 8  # 32
P = 128
CT = C // P  # 2 channel tiles
NQ = N // 2  # 2048 query rows per core
NBLK = 512  # n-block (query columns per block)
NBLKS = NQ // NBLK  # 4
MT = N // P  # 32 m-tiles (key/value positions)
CHUNK = 512
NCHUNKS = N // CHUNK  # 8
QCHUNKS = NQ // CHUNK  # 4
NCORES = 8

_prog = None


def _build(bench_reps=None):
    import contextlib

    import concourse.bacc as bacc
    import concourse.mybir as mybir
    import concourse.tile as tile

    f32 = mybir.dt.float32
    f32r = mybir.dt.float32r
    bf16 = mybir.dt.bfloat16
    Exp = mybir.ActivationFunctionType.Exp

    nc = bacc.Bacc("TRN2", target_bir_lowering=False, debug=False)

    xh_d = nc.dram_tensor("xh", [C, N], f32r, kind="ExternalInput")
    xq_d = nc.dram_tensor("xq", [C, NQ], f32r, kind="ExternalInput")
    # wqt/wkt are zero-padded on the host from [C, C8] to [C, 128] so the
    # projection matmul writes all 128 partitions of q/k (zero rows included)
    wqt_d = nc.dram_tensor("wqt", [C, P], f32r, kind="ExternalInput")
    wkt_d = nc.dram_tensor("wkt", [C, P], f32r, kind="ExternalInput")
    wvt_d = nc.dram_tensor("wvt", [C, C], f32r, kind="ExternalInput")
    bq_d = nc.dram_tensor("bq", [P], f32, kind="ExternalInput")
    bk_d = nc.dram_tensor("bk", [P], f32, kind="ExternalInput")
    bv_d = nc.dram_tensor("bv", [C], f32, kind="ExternalInput")
    g_d = nc.dram_tensor("gamma", [1], f32, kind="ExternalInput")
    out_d = nc.dram_tensor("out", [C, NQ], f32, kind="ExternalOutput")

    with tile.TileContext(nc) as tc:
        with (
            tc.tile_pool(name="const", bufs=1) as const,
            tc.tile_pool(name="big", bufs=1) as big,
        ):
            # persistent SBUF tensors
            xh = big.tile([P, CT, N], f32r)  # x[b], channels on partitions
            xq = big.tile([P, CT, NQ], f32r)  # this core's n-slice of x[b]
            k_sb = big.tile([P, N], f32r)  # k, zero rows 32..127
            q_sb = big.tile([P, NQ], f32r)  # q, zero rows 32..127
            vt_sb = big.tile([P, MT, C], bf16)  # v^T tiles [m, c]

            wqt = const.tile([P, CT, P], f32r)
            wkt = const.tile([P, CT, P], f32r)
            wvt = const.tile([P, CT, C], f32r)
            bq_sb = const.tile([P, 1], f32)
            bk_sb = const.tile([P, 1], f32)
            bvb = const.tile([P, C], f32)  # bv broadcast over partitions
            gam = const.tile([1, 1], f32)
            ones_bf = const.tile([P, 1], bf16)
            ones1 = const.tile([1, P], f32)

            nc.sync.dma_start(out=wqt, in_=wqt_d.ap().rearrange("(t p) o -> p t o", p=P))
            nc.sync.dma_start(out=wkt, in_=wkt_d.ap().rearrange("(t p) o -> p t o", p=P))
            nc.sync.dma_start(out=wvt, in_=wvt_d.ap().rearrange("(t p) o -> p t o", p=P))
            nc.sync.dma_start(out=bq_sb, in_=bq_d.ap()[:, None])
            nc.sync.dma_start(out=bk_sb, in_=bk_d.ap()[:, None])
            nc.gpsimd.dma_start(out=bvb, in_=bv_d.ap()[None, :].to_broadcast([P, C]))
            nc.sync.dma_start(out=gam, in_=g_d.ap()[:, None])
            nc.vector.memset(ones_bf, 1.0)
            nc.vector.memset(ones1, 1.0)

            xh_r = xh_d.ap().rearrange("(t p) n -> p t n", p=P)
            xq_r = xq_d.ap().rearrange("(t p) n -> p t n", p=P)
            out_r = out_d.ap().rearrange("(t p) n -> p t n", p=P)

            for ch in range(NCHUNKS):
                sl = slice(ch * CHUNK, (ch + 1) * CHUNK)
                nc.sync.dma_start(out=xh[:, :, sl], in_=xh_r[:, :, sl])
            for ch in range(QCHUNKS):
                sl = slice(ch * CHUNK, (ch + 1) * CHUNK)
                nc.sync.dma_start(out=xq[:, :, sl], in_=xq_r[:, :, sl])

            # ---- phase A: q/k/v projections ----
            with tc.tile_pool(name="pa_psum", bufs=2, space="PSUM") as pap:
                for ch in range(NCHUNKS):
                    sl = slice(ch * CHUNK, (ch + 1) * CHUNK)
                    kp = pap.tile([P, CHUNK], f32, tag="kq", name="kp")
                    for t in range(CT):
                        nc.tensor.matmul(
                            kp,
                            lhsT=wkt[:, t, :],
                            rhs=xh[:, t, sl],
                            start=(t == 0),
                            stop=(t == CT - 1),
                        )
                    nc.vector.tensor_scalar_add(k_sb[:, sl], kp, bk_sb)
                for ch in range(QCHUNKS):
                    sl = slice(ch * CHUNK, (ch + 1) * CHUNK)
                    qp = pap.tile([P, CHUNK], f32, tag="kq", name="qp")
                    for t in range(CT):
                        nc.tensor.matmul(
                            qp,
                            lhsT=wqt[:, t, :],
                            rhs=xq[:, t, sl],
                            start=(t == 0),
                            stop=(t == CT - 1),
                        )
                    nc.vector.tensor_scalar_add(q_sb[:, sl], qp, bq_sb)
                for mt in range(MT):
                    msl = slice(mt * P, (mt + 1) * P)
                    vp = pap.tile([P, C], f32, tag="v", name="vp")
                    for t in range(CT):
                        nc.tensor.matmul(
                            vp,
                            lhsT=xh[:, t, msl],
                            rhs=wvt[:, t, :],
                            start=(t == 0),
                            stop=(t == CT - 1),
                        )
                    # drain + bias + cast to bf16 in one DVE op
                    nc.vector.tensor_add(out=vt_sb[:, mt, :], in0=vp, in1=bvb)

            # ---- phase B: attention ----
            GRP = 2  # m-tiles per S^T psum group (2 banks)
            with (
                tc.tile_pool(name="st_psum", bufs=2, space="PSUM") as stp,
                tc.tile_pool(name="acc_psum", bufs=1, space="PSUM") as accp,
                tc.tile_pool(name="pt_pool", bufs=2) as ptp,
                tc.tile_pool(name="fin_pool", bufs=3) as finp,
            ):
                loop_ctx = (
                    tc.For_i(0, bench_reps, 1)
                    if bench_reps is not None
                    else contextlib.nullcontext()
                )
                with loop_ctx:
                    for nb in range(NBLKS):
                        nsl = slice(nb * NBLK, (nb + 1) * NBLK)
                        pt = ptp.tile([P, MT, NBLK], bf16, tag="pt", name="pt")
                        out_ps0 = accp.tile([P, NBLK], f32, tag="out0", name="out_ps0")
                        out_ps1 = accp.tile([P, NBLK], f32, tag="out1", name="out_ps1")
                        out_ps = [out_ps0, out_ps1]
                        den_ps = accp.tile([1, NBLK], f32, tag="den", name="den_ps")
                        for mg in range(MT // GRP):
                            st = stp.tile([P, GRP, NBLK], f32, tag="st", name="st")
                            for i in range(GRP):
                                mt = GRP * mg + i
                                nc.tensor.matmul(
                                    st[:, i, :],
                                    lhsT=k_sb[:, mt * P : (mt + 1) * P],
                                    rhs=q_sb[:, nsl],
                                    start=True,
                                    stop=True,
                                )
                            nc.scalar.activation(
                                out=pt[:, GRP * mg : GRP * (mg + 1), :],
                                in_=st,
                                func=Exp,
                            )
                            for i in range(GRP):
                                mt = GRP * mg + i
                                for cc in range(CT):
                                    nc.tensor.matmul(
                                        out_ps[cc],
                                        lhsT=vt_sb[:, mt, cc * P : (cc + 1) * P],
                                        rhs=pt[:, mt, :],
                                        start=(mt == 0),
                                        stop=(mt == MT - 1),
                                    )
                                nc.tensor.matmul(
                                    den_ps,
                                    lhsT=ones_bf,
                                    rhs=pt[:, mt, :],
                                    start=(mt == 0),
                                    stop=(mt == MT - 1),
                                )
                        # normalization: y = out * (gamma/denom) + x
                        rec = finp.tile([1, NBLK], f32, tag="rec", name="rec")
                        nc.vector.reciprocal(rec, den_ps)
                        nc.vector.tensor_scalar_mul(rec, rec, gam)
                        bc_ps = accp.tile([P, NBLK], f32, tag="bc", name="bc_ps")
                        # exact fp32 outer-product broadcast: bc[p, n] = rec[n]
                        nc.tensor.matmul(
                            bc_ps, lhsT=ones1, rhs=rec, start=True, stop=True
                        )
                        bc_sb = finp.tile([P, NBLK], f32, tag="bcs", name="bc_sb")
                        nc.vector.tensor_copy(out=bc_sb, in_=bc_ps)
                        for cc in range(CT):
                            fin = finp.tile([P, NBLK], f32, tag="fin", name="fin")
                            nc.vector.tensor_mul(out=fin, in0=out_ps[cc], in1=bc_sb)
                            nc.vector.tensor_add(
                                out=fin, in0=fin, in1=xq[:, cc, nsl].bitcast(f32)
                            )
                            nc.sync.dma_start(out=out_r[:, cc, nsl], in_=fin)

    nc.compile()
    return nc


def _get_prog():
    global _prog
    if _prog is None:
        _prog = _build()
    return _prog


def make_in_maps(inputs):
    x = np.ascontiguousarray(inputs["x"], dtype=np.float32).reshape(B, C, N)
    wqt = np.zeros((C, P), np.float32)
    wqt[:, :C8] = np.asarray(inputs["wq"], np.float32).T
    wkt = np.zeros((C, P), np.float32)
    wkt[:, :C8] = np.asarray(inputs["wk"], np.float32).T
    wvt = np.ascontiguousarray(np.asarray(inputs["wv"], np.float32).T)
    bq = np.zeros(P, np.float32)
    bq[:C8] = np.asarray(inputs["bq"], np.float32)
    bk = np.zeros(P, np.float32)
    bk[:C8] = np.asarray(inputs["bk"], np.float32)
    bv = np.ascontiguousarray(np.asarray(inputs["bv"], np.float32))
    gamma = np.ascontiguousarray(np.asarray(inputs["gamma"], np.float32).reshape(1))
    in_maps = []
    for core in range(NCORES):
        b, h = divmod(core, 2)
        in_maps.append(
            {
                "xh": x[b],
                "xq": np.ascontiguousarray(x[b][:, h * NQ : (h + 1) * NQ]),
                "wqt": wqt,
                "wkt": wkt,
                "wvt": wvt,
                "bq": bq,
                "bk": bk,
                "bv": bv,
                "gamma": gamma,
            }
        )
    return in_maps


def assemble(results):
    out = np.empty((B, C, N), np.float32)
    for core in range(NCORES):
        b, h = divmod(core, 2)
        out[b][:, h * NQ : (h + 1) * NQ] = results[core]["out"]
    return out.reshape(B, C, HH, WW)


def kernel(**inputs):
    from concourse.bass_utils import run_bass_kernel_spmd

    nc = _get_prog()
    in_maps = make_in_maps(inputs)
    res = run_bass_kernel_spmd(nc, in_maps, core_ids=list(range(NCORES)))
    return assemble(res.results)


# revision 6
# speedup vs baseline: 9413.9002x; 9.6599x over previous
"""SAGAN-style attention block (nn_AttentionBlock) on 8 Trainium2 NeuronCores.

Math (per batch b):
    q = wq @ x + bq            [C8, N]
    k = wk @ x + bk            [C8, N]
    v = wv @ x + bv            [C,  N]
    S[n, m]  = sum_o q[o,n] k[o,m]
    attn     = softmax_m(S)
    out[c,n] = sum_m v[c,m] attn[n,m]
    y        = gamma * out + x

Sharding: 8 cores = 4 batches x 2 halves of the n (query-row) axis.  Each
core holds the full x[b] (for K/V) plus its own n-slice (for Q + residual).

Per-core layout (channels/rows on partitions):
    S^T tiles [m(128 part), n(512)] via matmul(lhsT=k_tile, rhs=q_block)
    P^T = exp(S^T)  (no max subtraction: |S| <~ 40, safe in fp32/bf16)
    out[c,n]   = sum over 32 m-tiles of matmul(lhsT=vT[m,c], rhs=P^T[m,n])
    denom[1,n] = matmul(lhsT=ones[128,1], rhs=P^T)  accumulated the same way
    y = out * (gamma/denom broadcast) + x

QK^T/projection matmuls run as float32r (full-rate fp32 PE mode); the
AV/denominator path runs in bf16 (P^T is written by Exp directly as bf16).
"""

import sys

sys.path.insert(0, "/opt/trn_rl_repo")

import numpy as np  # noqa: E402

B, C, HH, WW = 4, 256, 64, 64
N = HH * WW  # 4096
C8 = C // 8  # 32
P = 128
CT = C // P  # 2 channel tiles
NQ = N // 2  # 2048 query rows per core
NBLK = 512  # n-block (query columns per block)
NBLKS = NQ // NBLK  # 4
MT = N // P  # 32 m-tiles (key/value positions)
CHUNK = 512
NCHUNKS = N // CHUNK  # 8
QCHUNKS = NQ // CHUNK  # 4
NCORES = 8

_prog = None


def _build(bench_reps=None, variant="full"):
    import contextlib

    import concourse.bacc as bacc
    import concourse.mybir as mybir
    import concourse.tile as tile

    f32 = mybir.dt.float32
    f32r = mybir.dt.float32r
    bf16 = mybir.dt.bfloat16
    Exp = mybir.ActivationFunctionType.Exp

    nc = bacc.Bacc("TRN2", target_bir_lowering=False, debug=False)

    xh_d = nc.dram_tensor("xh", [C, N], f32r, kind="ExternalInput")
    xq_d = nc.dram_tensor("xq", [C, NQ], f32r, kind="ExternalInput")
    # wqt/wkt are zero-padded on the host from [C, C8] to [C, 128] so the
    # projection matmul writes all 128 partitions of q/k (zero rows included)
    wqt_d = nc.dram_tensor("wqt", [C, P], f32r, kind="ExternalInput")
    wkt_d = nc.dram_tensor("wkt", [C, P], f32r, kind="ExternalInput")
    wvt_d = nc.dram_tensor("wvt", [C, C], f32r, kind="ExternalInput")
    bq_d = nc.dram_tensor("bq", [P], f32, kind="ExternalInput")
    bk_d = nc.dram_tensor("bk", [P], f32, kind="ExternalInput")
    bv_d = nc.dram_tensor("bv", [C], f32, kind="ExternalInput")
    g_d = nc.dram_tensor("gamma", [1], f32, kind="ExternalInput")
    out_d = nc.dram_tensor("out", [C, NQ], f32, kind="ExternalOutput")

    with tile.TileContext(nc) as tc:
        with (
            tc.tile_pool(name="const", bufs=1) as const,
            tc.tile_pool(name="big", bufs=1) as big,
        ):
            # persistent SBUF tensors
            xh = big.tile([P, CT, N], f32r)  # x[b], channels on partitions
            xq = big.tile([P, CT, NQ], f32r)  # this core's n-slice of x[b]
            k_sb = big.tile([P, N], f32r)  # k, zero rows 32..127
            q_sb = big.tile([P, NQ], f32r)  # q, zero rows 32..127
            vt_sb = big.tile([P, MT, C], bf16)  # v^T tiles [m, c]

            wqt = const.tile([P, CT, P], f32r)
            wkt = const.tile([P, CT, P], f32r)
            wvt = const.tile([P, CT, C], f32r)
            bq_sb = const.tile([P, 1], f32)
            bk_sb = const.tile([P, 1], f32)
            bvb = const.tile([P, C], f32)  # bv broadcast over partitions
            gam = const.tile([1, 1], f32)
            ones_bf = const.tile([P, 1], bf16)
            ones1 = const.tile([1, P], f32)

            nc.sync.dma_start(out=wqt, in_=wqt_d.ap().rearrange("(t p) o -> p t o", p=P))
            nc.sync.dma_start(out=wkt, in_=wkt_d.ap().rearrange("(t p) o -> p t o", p=P))
            nc.sync.dma_start(out=wvt, in_=wvt_d.ap().rearrange("(t p) o -> p t o", p=P))
            nc.sync.dma_start(out=bq_sb, in_=bq_d.ap()[:, None])
            nc.sync.dma_start(out=bk_sb, in_=bk_d.ap()[:, None])
            nc.gpsimd.dma_start(out=bvb, in_=bv_d.ap()[None, :].to_broadcast([P, C]))
            nc.sync.dma_start(out=gam, in_=g_d.ap()[:, None])
            nc.vector.memset(ones_bf, 1.0)
            nc.vector.memset(ones1, 1.0)

            xh_r = xh_d.ap().rearrange("(t p) n -> p t n", p=P)
            xq_r = xq_d.ap().rearrange("(t p) n -> p t n", p=P)
            out_r = out_d.ap().rearrange("(t p) n -> p t n", p=P)

            for ch in range(NCHUNKS):
                sl = slice(ch * CHUNK, (ch + 1) * CHUNK)
                nc.sync.dma_start(out=xh[:, :, sl], in_=xh_r[:, :, sl])
            for ch in range(QCHUNKS):
                sl = slice(ch * CHUNK, (ch + 1) * CHUNK)
                nc.sync.dma_start(out=xq[:, :, sl], in_=xq_r[:, :, sl])

            # ---- phase A: q/k/v projections ----
            with tc.tile_pool(name="pa_psum", bufs=2, space="PSUM") as pap:
                for ch in range(NCHUNKS):
                    sl = slice(ch * CHUNK, (ch + 1) * CHUNK)
                    kp = pap.tile([P, CHUNK], f32, tag="kq", name="kp")
                    for t in range(CT):
                        nc.tensor.matmul(
                            kp,
                            lhsT=wkt[:, t, :],
                            rhs=xh[:, t, sl],
                            start=(t == 0),
                            stop=(t == CT - 1),
                        )
                    nc.vector.tensor_scalar_add(k_sb[:, sl], kp, bk_sb)
                for ch in range(QCHUNKS):
                    sl = slice(ch * CHUNK, (ch + 1) * CHUNK)
                    qp = pap.tile([P, CHUNK], f32, tag="kq", name="qp")
                    for t in range(CT):
                        nc.tensor.matmul(
                            qp,
                            lhsT=wqt[:, t, :],
                            rhs=xq[:, t, sl],
                            start=(t == 0),
                            stop=(t == CT - 1),
                        )
                    nc.vector.tensor_scalar_add(q_sb[:, sl], qp, bq_sb)
                for mt in range(MT):
                    msl = slice(mt * P, (mt + 1) * P)
                    vp = pap.tile([P, C], f32, tag="v", name="vp")
                    for t in range(CT):
                        nc.tensor.matmul(
                            vp,
                            lhsT=xh[:, t, msl],
                            rhs=wvt[:, t, :],
                            start=(t == 0),
                            stop=(t == CT - 1),
                        )
                    # drain + bias + cast to bf16 in one DVE op
                    nc.vector.tensor_add(out=vt_sb[:, mt, :], in0=vp, in1=bvb)

            # ---- phase B: attention ----
            GRP = 2  # m-tiles per S^T psum group (2 banks)
            with (
                tc.tile_pool(name="st_psum", bufs=2, space="PSUM") as stp,
                tc.tile_pool(name="acc_psum", bufs=1, space="PSUM") as accp,
                tc.tile_pool(name="pt_pool", bufs=2) as ptp,
                tc.tile_pool(name="fin_pool", bufs=3) as finp,
            ):
                loop_ctx = (
                    tc.For_i(0, bench_reps, 1)
                    if bench_reps is not None
                    else contextlib.nullcontext()
                )
                with loop_ctx:
                    for nb in range(NBLKS):
                        nsl = slice(nb * NBLK, (nb + 1) * NBLK)
                        pt = ptp.tile([P, MT, NBLK], bf16, tag="pt", name="pt")
                        out_ps0 = accp.tile([P, NBLK], f32, tag="out0", name="out_ps0")
                        out_ps1 = accp.tile([P, NBLK], f32, tag="out1", name="out_ps1")
                        out_ps = [out_ps0, out_ps1]
                        den_ps = accp.tile([1, NBLK], f32, tag="den", name="den_ps")
                        for mg in range(MT // GRP):
                            st = stp.tile([P, GRP, NBLK], f32, tag="st", name="st")
                            if variant in ("full", "qk"):
                                for i in range(GRP):
                                    mt = GRP * mg + i
                                    nc.tensor.matmul(
                                        st[:, i, :],
                                        lhsT=k_sb[:, mt * P : (mt + 1) * P],
                                        rhs=q_sb[:, nsl],
                                        start=True,
                                        stop=True,
                                    )
                                nc.scalar.activation(
                                    out=pt[:, GRP * mg : GRP * (mg + 1), :],
                                    in_=st,
                                    func=Exp,
                                )
                            if variant in ("full", "av"):
                                for i in range(GRP):
                                    mt = GRP * mg + i
                                    for cc in range(CT):
                                        nc.tensor.matmul(
                                            out_ps[cc],
                                            lhsT=vt_sb[:, mt, cc * P : (cc + 1) * P],
                                            rhs=pt[:, mt, :],
                                            start=(mt == 0),
                                            stop=(mt == MT - 1),
                                        )
                                    nc.tensor.matmul(
                                        den_ps,
                                        lhsT=ones_bf,
                                        rhs=pt[:, mt, :],
                                        start=(mt == 0),
                                        stop=(mt == MT - 1),
                                    )
                        if variant != "full":
                            continue
                        # normalization: y = out * (gamma/denom) + x
                        rec = finp.tile([1, NBLK], f32, tag="rec", name="rec")
                        nc.vector.reciprocal(rec, den_ps)
                        nc.vector.tensor_scalar_mul(rec, rec, gam)
                        bc_ps = accp.tile([P, NBLK], f32, tag="bc", name="bc_ps")
                        # exact fp32 outer-product broadcast: bc[p, n] = rec[n]
                        nc.tensor.matmul(
                            bc_ps, lhsT=ones1, rhs=rec, start=True, stop=True
                        )
                        bc_sb = finp.tile([P, NBLK], f32, tag="bcs", name="bc_sb")
                        nc.vector.tensor_copy(out=bc_sb, in_=bc_ps)
                        for cc in range(CT):
                            fin = finp.tile([P, NBLK], f32, tag="fin", name="fin")
                            nc.vector.tensor_mul(out=fin, in0=out_ps[cc], in1=bc_sb)
                            nc.vector.tensor_add(
                                out=fin, in0=fin, in1=xq[:, cc, nsl].bitcast(f32)
                            )
                            nc.sync.dma_start(out=out_r[:, cc, nsl], in_=fin)

    nc.compile()
    return nc


def _get_prog():
    global _prog
    if _prog is None:
        _prog = _build()
    return _prog


def make_in_maps(inputs):
    x = np.ascontiguousarray(inputs["x"], dtype=np.float32).reshape(B, C, N)
    wqt = np.zeros((C, P), np.float32)
    wqt[:, :C8] = np.asarray(inputs["wq"], np.float32).T
    wkt = np.zeros((C, P), np.float32)
    wkt[:, :C8] = np.asarray(inputs["wk"], np.float32).T
    wvt = np.ascontiguousarray(np.asarray(inputs["wv"], np.float32).T)
    bq = np.zeros(P, np.float32)
    bq[:C8] = np.asarray(inputs["bq"], np.float32)
    bk = np.zeros(P, np.float32)
    bk[:C8] = np.asarray(inputs["bk"], np.float32)
    bv = np.ascontiguousarray(np.asarray(inputs["bv"], np.float32))
    gamma = np.ascontiguousarray(np.asarray(inputs["gamma"], np.float32).reshape(1))
    in_maps = []
    for core in range(NCORES):
        b, h = divmod(core, 2)
        in_maps.append(
            {
                "xh": x[b],
                "xq": np.ascontiguousarray(x[b][:, h * NQ : (h + 1) * NQ]),
                "wqt": wqt,
                "wkt": wkt,
                "wvt": wvt,
                "bq": bq,
                "bk": bk,
                "bv": bv,
                "gamma": gamma,
            }
        )
    return in_maps


def assemble(results):
    out = np.empty((B, C, N), np.float32)
    for core in range(NCORES):
        b, h = divmod(core, 2)
        out[b][:, h * NQ : (h + 1) * NQ] = results[core]["out"]
    return out.reshape(B, C, HH, WW)


def kernel(**inputs):
    from concourse.bass_utils import run_bass_kernel_spmd

    nc = _get_prog()
    in_maps = make_in_maps(inputs)
    res = run_bass_kernel_spmd(nc, in_maps, core_ids=list(range(NCORES)))
    return assemble(res.results)


# revision 8
# speedup vs baseline: 9689.5659x; 1.0293x over previous
"""SAGAN-style attention block (nn_AttentionBlock) on 8 Trainium2 NeuronCores.

Math (per batch b):
    q = wq @ x + bq            [C8, N]
    k = wk @ x + bk            [C8, N]
    v = wv @ x + bv            [C,  N]
    S[n, m]  = sum_o q[o,n] k[o,m]
    attn     = softmax_m(S)
    out[c,n] = sum_m v[c,m] attn[n,m]
    y        = gamma * out + x

Sharding: 8 cores = 4 batches x 2 halves of the n (query-row) axis.  Each
core holds the full x[b] (for K/V) plus its own n-slice (for Q + residual).

Per-core layout (channels/rows on partitions):
    S^T tiles [m(128 part), n(512)] via matmul(lhsT=k_tile, rhs=q_block)
    P^T = exp(S^T)  (no max subtraction: |S| <~ 40, safe in fp32/bf16)
    out[c,n]   = sum over 32 m-tiles of matmul(lhsT=vT[m,c], rhs=P^T[m,n])
    denom[1,n] = matmul(lhsT=ones[128,1], rhs=P^T)  accumulated the same way
    y = out * (gamma/denom broadcast) + x

QK^T/projection matmuls run as float32r (full-rate fp32 PE mode); the
AV/denominator path runs in bf16 (P^T is written by Exp directly as bf16).
"""

import sys

sys.path.insert(0, "/opt/trn_rl_repo")

import numpy as np  # noqa: E402

B, C, HH, WW = 4, 256, 64, 64
N = HH * WW  # 4096
C8 = C // 8  # 32
P = 128
CT = C // P  # 2 channel tiles
NQ = N // 2  # 2048 query rows per core
NBLK = 512  # n-block (query columns per block)
NBLKS = NQ // NBLK  # 4
MT = N // P  # 32 m-tiles (key/value positions)
CHUNK = 512
NCHUNKS = N // CHUNK  # 8
QCHUNKS = NQ // CHUNK  # 4
NCORES = 8

_prog = None


def _build(bench_reps=None, variant="full"):
    import contextlib

    import concourse.bacc as bacc
    import concourse.mybir as mybir
    import concourse.tile as tile

    f32 = mybir.dt.float32
    f32r = mybir.dt.float32r
    bf16 = mybir.dt.bfloat16
    Exp = mybir.ActivationFunctionType.Exp

    nc = bacc.Bacc("TRN2", target_bir_lowering=False, debug=False)

    xh_d = nc.dram_tensor("xh", [C, N], f32r, kind="ExternalInput")
    xq_d = nc.dram_tensor("xq", [C, NQ], f32r, kind="ExternalInput")
    # wqt/wkt are zero-padded on the host from [C, C8] to [C, 128] so the
    # projection matmul writes all 128 partitions of q/k (zero rows included)
    wqt_d = nc.dram_tensor("wqt", [C, P], f32r, kind="ExternalInput")
    wkt_d = nc.dram_tensor("wkt", [C, P], f32r, kind="ExternalInput")
    wvt_d = nc.dram_tensor("wvt", [C, C], f32r, kind="ExternalInput")
    bq_d = nc.dram_tensor("bq", [P], f32, kind="ExternalInput")
    bk_d = nc.dram_tensor("bk", [P], f32, kind="ExternalInput")
    bv_d = nc.dram_tensor("bv", [C], f32, kind="ExternalInput")
    g_d = nc.dram_tensor("gamma", [1], f32, kind="ExternalInput")
    out_d = nc.dram_tensor("out", [C, NQ], f32, kind="ExternalOutput")

    with tile.TileContext(nc) as tc:
        with (
            tc.tile_pool(name="const", bufs=1) as const,
            tc.tile_pool(name="big", bufs=1) as big,
        ):
            # persistent SBUF tensors
            xh = big.tile([P, CT, N], f32r)  # x[b], channels on partitions
            xq = big.tile([P, CT, NQ], f32r)  # this core's n-slice of x[b]
            k_sb = big.tile([P, N], f32r)  # k, zero rows 32..127
            q_sb = big.tile([P, NQ], f32r)  # q rows 0..31; replicated to all
            # 4 partition groups for row-packed QK^T
            k_pk = big.tile([P, MT // 4, P], f32r)  # k m-tile 4s+g at [32g:, s, :]
            vt_sb = big.tile([P, MT, C], bf16)  # v^T tiles [m, c]

            wqt = const.tile([P, CT, P], f32r)
            wkt = const.tile([P, CT, P], f32r)
            wvt = const.tile([P, CT, C], f32r)
            bq_sb = const.tile([P, 1], f32)
            bk_sb = const.tile([P, 1], f32)
            bvb = const.tile([P, C], f32)  # bv broadcast over partitions
            gam = const.tile([1, 1], f32)
            ones_bf = const.tile([P, 1], bf16)
            ones1 = const.tile([1, P], f32)

            nc.sync.dma_start(out=wqt, in_=wqt_d.ap().rearrange("(t p) o -> p t o", p=P))
            nc.sync.dma_start(out=wkt, in_=wkt_d.ap().rearrange("(t p) o -> p t o", p=P))
            nc.sync.dma_start(out=wvt, in_=wvt_d.ap().rearrange("(t p) o -> p t o", p=P))
            nc.sync.dma_start(out=bq_sb, in_=bq_d.ap()[:, None])
            nc.sync.dma_start(out=bk_sb, in_=bk_d.ap()[:, None])
            nc.gpsimd.dma_start(out=bvb, in_=bv_d.ap()[None, :].to_broadcast([P, C]))
            nc.sync.dma_start(out=gam, in_=g_d.ap()[:, None])
            nc.vector.memset(ones_bf, 1.0)
            nc.vector.memset(ones1, 1.0)

            xh_r = xh_d.ap().rearrange("(t p) n -> p t n", p=P)
            xq_r = xq_d.ap().rearrange("(t p) n -> p t n", p=P)
            out_r = out_d.ap().rearrange("(t p) n -> p t n", p=P)

            for ch in range(NCHUNKS):
                sl = slice(ch * CHUNK, (ch + 1) * CHUNK)
                nc.sync.dma_start(out=xh[:, :, sl], in_=xh_r[:, :, sl])
            for ch in range(QCHUNKS):
                sl = slice(ch * CHUNK, (ch + 1) * CHUNK)
                nc.sync.dma_start(out=xq[:, :, sl], in_=xq_r[:, :, sl])

            # ---- phase A: q/k/v projections ----
            with tc.tile_pool(name="pa_psum", bufs=2, space="PSUM") as pap:
                for ch in range(NCHUNKS):
                    sl = slice(ch * CHUNK, (ch + 1) * CHUNK)
                    kp = pap.tile([P, CHUNK], f32, tag="kq", name="kp")
                    for t in range(CT):
                        nc.tensor.matmul(
                            kp,
                            lhsT=wkt[:, t, :],
                            rhs=xh[:, t, sl],
                            start=(t == 0),
                            stop=(t == CT - 1),
                        )
                    nc.vector.tensor_scalar_add(k_sb[:, sl], kp, bk_sb)
                    for g in range(4):
                        mt = 4 * ch + g
                        nc.sync.dma_start(
                            out=k_pk[32 * g : 32 * g + 32, ch, :],
                            in_=k_sb[:C8, mt * P : (mt + 1) * P],
                        )
                for ch in range(QCHUNKS):
                    sl = slice(ch * CHUNK, (ch + 1) * CHUNK)
                    qp = pap.tile([P, CHUNK], f32, tag="kq", name="qp")
                    for t in range(CT):
                        nc.tensor.matmul(
                            qp,
                            lhsT=wqt[:, t, :],
                            rhs=xq[:, t, sl],
                            start=(t == 0),
                            stop=(t == CT - 1),
                        )
                    nc.vector.tensor_scalar_add(q_sb[:, sl], qp, bq_sb)
                    for g in range(1, 4):
                        nc.sync.dma_start(
                            out=q_sb[32 * g : 32 * g + 32, sl], in_=q_sb[:C8, sl]
                        )
                for mt in range(MT):
                    msl = slice(mt * P, (mt + 1) * P)
                    vp = pap.tile([P, C], f32, tag="v", name="vp")
                    for t in range(CT):
                        nc.tensor.matmul(
                            vp,
                            lhsT=xh[:, t, msl],
                            rhs=wvt[:, t, :],
                            start=(t == 0),
                            stop=(t == CT - 1),
                        )
                    # drain + bias + cast to bf16 in one DVE op
                    nc.vector.tensor_add(out=vt_sb[:, mt, :], in0=vp, in1=bvb)

            # ---- phase B: attention ----
            GRP = 4  # m-tiles per S^T psum group (4 banks, one per row group)
            with (
                tc.tile_pool(name="st_psum", bufs=1, space="PSUM") as stp,
                tc.tile_pool(name="acc_psum", bufs=1, space="PSUM") as accp,
                tc.tile_pool(name="pt_pool", bufs=2) as ptp,
                tc.tile_pool(name="fin_pool", bufs=3) as finp,
            ):
                loop_ctx = (
                    tc.For_i(0, bench_reps, 1)
                    if bench_reps is not None
                    else contextlib.nullcontext()
                )
                with loop_ctx:
                    NSLOT = MT // GRP  # 16 S^T/exp slots per block
                    bstate = {}  # nb -> (nsl, pt, out_ps, den_ps)

                    def emit_av(nb, mg):
                        """AV + denominator matmuls for slot (nb, mg)."""
                        if variant not in ("full", "av"):
                            return
                        nsl, pt, out_ps, den_ps = bstate[nb]
                        for i in range(GRP):
                            mt = GRP * mg + i
                            for cc in range(CT):
                                nc.tensor.matmul(
                                    out_ps[cc],
                                    lhsT=vt_sb[:, mt, cc * P : (cc + 1) * P],
                                    rhs=pt[:, mt, :],
                                    start=(mt == 0),
                                    stop=(mt == MT - 1),
                                )
                            nc.tensor.matmul(
                                den_ps,
                                lhsT=ones_bf,
                                rhs=pt[:, mt, :],
                                start=(mt == 0),
                                stop=(mt == MT - 1),
                            )

                    def emit_tail(nb):
                        """Drain PSUM eagerly, then normalize + residual + store.

                        y = out * (gamma/denom) + x.  The PSUM->SBUF copies come
                        first so the accumulator banks free up for the next
                        block's matmuls; the rest overlaps next-block PE work.
                        """
                        if variant != "full":
                            return
                        nsl, pt, out_ps, den_ps = bstate.pop(nb)
                        rec = finp.tile([1, NBLK], f32, tag="rec", name="rec")
                        nc.vector.reciprocal(rec, den_ps)
                        nc.vector.tensor_scalar_mul(rec, rec, gam)
                        outc = []
                        for cc in range(CT):
                            oc = finp.tile([P, NBLK], f32, tag=f"oc{cc}", name="oc")
                            nc.vector.tensor_copy(out=oc, in_=out_ps[cc])
                            outc.append(oc)
                        bc_sb = finp.tile([P, NBLK], f32, tag="bcs", name="bc_sb")
                        nc.gpsimd.partition_broadcast(bc_sb, rec)
                        for cc in range(CT):
                            fin = finp.tile([P, NBLK], f32, tag="fin", name="fin")
                            nc.vector.tensor_mul(out=fin, in0=outc[cc], in1=bc_sb)
                            nc.vector.tensor_add(
                                out=fin, in0=fin, in1=xq[:, cc, nsl].bitcast(f32)
                            )
                            nc.sync.dma_start(out=out_r[:, cc, nsl], in_=fin)

                    # software-pipelined emission: the AV/den matmuls for slot
                    # s-1 are emitted between S^T(s) and its exp, so the PE
                    # never sits idle waiting for the ACT engine's exp
                    prev = None
                    for nb in range(NBLKS):
                        nsl = slice(nb * NBLK, (nb + 1) * NBLK)
                        pt = ptp.tile([P, MT, NBLK], bf16, tag="pt", name="pt")
                        out_ps0 = accp.tile([P, NBLK], f32, tag="out0", name="out_ps0")
                        out_ps1 = accp.tile([P, NBLK], f32, tag="out1", name="out_ps1")
                        den_ps = accp.tile([1, NBLK], f32, tag="den", name="den_ps")
                        bstate[nb] = (nsl, pt, [out_ps0, out_ps1], den_ps)
                        for mg in range(NSLOT):
                            if variant in ("full", "qk"):
                                st = stp.tile([P, GRP, NBLK], f32, tag="st", name="st")
                                for g in range(GRP):
                                    nc.tensor.matmul(
                                        st[:, g, :],
                                        lhsT=k_pk[32 * g : 32 * g + 32, mg, :],
                                        rhs=q_sb[32 * g : 32 * g + 32, nsl],
                                        start=True,
                                        stop=True,
                                        tile_position=(32 * g, 0),
                                    )
                                nc.scalar.activation(
                                    out=pt[:, GRP * mg : GRP * (mg + 1), :],
                                    in_=st,
                                    func=Exp,
                                )
                            if prev is not None:
                                pnb, pmg = prev
                                emit_av(pnb, pmg)
                                if pmg == NSLOT - 1:
                                    emit_tail(pnb)
                            prev = (nb, mg)
                    if prev is not None:
                        pnb, pmg = prev
                        emit_av(pnb, pmg)
                        emit_tail(pnb)

    nc.compile()
    return nc


def _get_prog():
    global _prog
    if _prog is None:
        _prog = _build()
    return _prog


def make_in_maps(inputs):
    x = np.ascontiguousarray(inputs["x"], dtype=np.float32).reshape(B, C, N)
    wqt = np.zeros((C, P), np.float32)
    wqt[:, :C8] = np.asarray(inputs["wq"], np.float32).T
    wkt = np.zeros((C, P), np.float32)
    wkt[:, :C8] = np.asarray(inputs["wk"], np.float32).T
    wvt = np.ascontiguousarray(np.asarray(inputs["wv"], np.float32).T)
    bq = np.zeros(P, np.float32)
    bq[:C8] = np.asarray(inputs["bq"], np.float32)
    bk = np.zeros(P, np.float32)
    bk[:C8] = np.asarray(inputs["bk"], np.float32)
    bv = np.ascontiguousarray(np.asarray(inputs["bv"], np.float32))
    gamma = np.ascontiguousarray(np.asarray(inputs["gamma"], np.float32).reshape(1))
    in_maps = []
    for core in range(NCORES):
        b, h = divmod(core, 2)
        in_maps.append(
            {
                "xh": x[b],
                "xq": np.ascontiguousarray(x[b][:, h * NQ : (h + 1) * NQ]),
                "wqt": wqt,
                "wkt": wkt,
                "wvt": wvt,
                "bq": bq,
                "bk": bk,
                "bv": bv,
                "gamma": gamma,
            }
        )
    return in_maps


def assemble(results):
    out = np.empty((B, C, N), np.float32)
    for core in range(NCORES):
        b, h = divmod(core, 2)
        out[b][:, h * NQ : (h + 1) * NQ] = results[core]["out"]
    return out.reshape(B, C, HH, WW)


def kernel(**inputs):
    from concourse.bass_utils import run_bass_kernel_spmd

    nc = _get_prog()
    in_maps = make_in_maps(inputs)
    res = run_bass_kernel_spmd(nc, in_maps, core_ids=list(range(NCORES)))
    return assemble(res.results)
